# revision 1
# baseline (speedup 1.0000x reference)
"""CRF log-loss kernel for TRN2, data-parallel over batch on 8 NeuronCores.

Algorithm (per core, 128 examples):
  Forward algorithm in the exp domain:
      u_{s+1}[j,b] = (sum_k exp(trans[j,k] + LN_SCALE) * u_s[k,b]) * exp(feat[b,s,j] + beta)
  One 64x65 stationary-weight matmul (65th row = column sums, used for
  renormalization feedback) + one vector multiply per step. Periodic
  per-example renormalization is applied as a per-partition bias inside the
  bulk exp(feats) on the scalar engine, with an exponent-extract rough log
  on the vector engine as feedback; exact log accounting happens once at
  the end. Gold-path score via iota==tag masks (emission) and gpsimd
  ap_gather from a replicated transition table (transition score).
"""
import numpy as np
import ml_dtypes
from contextlib import ExitStack

import concourse.bass as bass
import concourse.bacc as bacc
import concourse.tile as tile
import concourse.mybir as mybir
from concourse.bass_utils import run_bass_kernel_spmd

bf16 = ml_dtypes.bfloat16
f32 = mybir.dt.float32
bf16d = mybir.dt.bfloat16
i16 = mybir.dt.int16
u16 = mybir.dt.uint16
i32 = mybir.dt.int32

B, S, T = 1024, 512, 64
NC = 8
BC = B // NC            # 128 examples per core
CHUNK = 8               # steps per renorm/exp chunk
NCH = S // CHUNK        # 64 chunks
LAG = 2                 # controller application lag (in chunks)
LN_SCALE = -4.7         # mean drift folded into PA
LN2 = float(np.log(2.0))

AF = mybir.ActivationFunctionType
ALU = mybir.AluOpType
AXX = mybir.AxisListType.X


def _build_program():
    nc = bacc.Bacc("TRN2", target_bir_lowering=False, debug=False, num_devices=NC)

    feats_d = nc.dram_tensor("feats", [BC, S, T], f32, kind="ExternalInput")
    u0_d = nc.dram_tensor("u0", [T, BC], bf16d, kind="ExternalInput")
    pa_d = nc.dram_tensor("pa", [T, T + 1], bf16d, kind="ExternalInput")
    pfin_d = nc.dram_tensor("pfin", [T, 1], bf16d, kind="ExternalInput")
    hmask_d = nc.dram_tensor("hmask", [BC, S, T], bf16d, kind="ExternalInput")
    startw_d = nc.dram_tensor("startw", [BC, T], f32, kind="ExternalInput")
    transrep_d = nc.dram_tensor("transrep", [BC, T * T], f32, kind="ExternalInput")
    pairsw_d = nc.dram_tensor("pairsw", [BC, 16 * 32], u16, kind="ExternalInput")
    m16_d = nc.dram_tensor("m16", [BC, 16], bf16d, kind="ExternalInput")
    out_d = nc.dram_tensor("out", [BC, 1], f32, kind="ExternalOutput")

    with tile.TileContext(nc) as tc, ExitStack() as ctx:
        cpool = ctx.enter_context(tc.tile_pool(name="const", bufs=1))
        fpool = ctx.enter_context(tc.tile_pool(name="feats", bufs=3))
        epool = ctx.enter_context(tc.tile_pool(name="ech", bufs=3))
        etpool = ctx.enter_context(tc.tile_pool(name="ett", bufs=8))
        upool = ctx.enter_context(tc.tile_pool(name="u", bufs=4))
        pspool = ctx.enter_context(tc.tile_pool(name="ps", bufs=4, space="PSUM"))
        ps2pool = ctx.enter_context(tc.tile_pool(name="ps2", bufs=1, space="PSUM"))
        bhpool = ctx.enter_context(tc.tile_pool(name="bh", bufs=4))
        mpool = ctx.enter_context(tc.tile_pool(name="mask", bufs=2))
        scpool = ctx.enter_context(tc.tile_pool(name="scratch", bufs=2))
        gpool = ctx.enter_context(tc.tile_pool(name="gather", bufs=2))

        # ---- constants into SBUF ----
        pa_s = cpool.tile([T, T + 1], bf16d)
        nc.sync.dma_start(pa_s[:, :], pa_d[:, :])
        pfin_s = cpool.tile([T, 1], bf16d)
        nc.sync.dma_start(pfin_s[:, :], pfin_d[:, :])
        h0_s = cpool.tile([BC, T], bf16d)
        nc.sync.dma_start(h0_s[:, :], hmask_d[:, 0, :])
        hL_s = cpool.tile([BC, T], bf16d)
        nc.sync.dma_start(hL_s[:, :], hmask_d[:, S - 1, :])
        startw_s = cpool.tile([BC, T], f32)
        nc.sync.dma_start(startw_s[:, :], startw_d[:, :])
        transrep_s = cpool.tile([BC, T * T], f32)
        nc.sync.dma_start(transrep_s[:, :], transrep_d[:, :])
        pairsw_s = cpool.tile([BC, 16 * 32], u16)
        nc.sync.dma_start(pairsw_s[:, :], pairsw_d[:, :])
        m16_s = cpool.tile([BC, 16], bf16d)
        nc.sync.dma_start(m16_s[:, :], m16_d[:, :])

        id1 = cpool.tile([1, 1], f32)
        nc.vector.memset(id1[:, :], 1.0)
        zcol = cpool.tile([BC, 1], f32)
        nc.vector.memset(zcol[:, :], 0.0)

        zrow = cpool.tile([1, BC], f32)
        nc.vector.memset(zrow[:, :], 0.0)

        # emission partial sums, one column per chunk
        parts = cpool.tile([BC, NCH], f32)
        # gathered-transition reduction columns, one per gather call
        rt16 = cpool.tile([BC, 16], f32)

        ucur = upool.tile([T, BC], bf16d)
        nc.sync.dma_start(ucur[:, :], u0_d[:, :])

        # ---- gold: transition-score gathers (independent of the chain) ----
        # priming copies: pool instructions encode at most ONE sync wait, so
        # make gpsimd observe each input tile one at a time up front
        pr1 = scpool.tile([BC, 1], f32)
        nc.gpsimd.tensor_copy(pr1[:, :], transrep_s[:, 0:1])
        pr2 = scpool.tile([BC, 1], u16)
        nc.gpsimd.tensor_copy(pr2[:, :], pairsw_s[:, 0:1])
        for i in range(16):
            gout = gpool.tile([BC, 512], f32)
            nc.gpsimd.indirect_copy(
                gout[:, :].unsqueeze(-1),
                transrep_s[:, :],
                pairsw_s[:, i * 32:(i + 1) * 32],
                i_know_ap_gather_is_preferred=True,
            )
            nc.vector.tensor_reduce(rt16[:, i:i + 1], gout[:, 0:511], axis=AXX, op=ALU.add)

        # ---- main loop ----
        biases = []  # per-chunk ACT bias tiles
        bprev = zrow
        grow = zrow
        for t in range(NCH):
            fch = fpool.tile([BC, CHUNK, T], f32)
            nc.sync.dma_start(fch[:, :, :], feats_d[:, t * CHUNK:(t + 1) * CHUNK, :])

            bias_ap = zcol[:, :] if t < LAG else biases[t - LAG]
            ech = epool.tile([BC, CHUNK * T], bf16d)
            nc.scalar.activation(ech[:, :], fch[:, :, :].rearrange("p a b -> p (a b)"),
                                 AF.Exp, bias=bias_ap, scale=1.0)

            # transpose E to [(s,j), b] in pairs of steps via DMA xbar
            etts = []
            for p in range(CHUNK // 2):
                ett = etpool.tile([2 * T, BC], bf16d)
                nc.sync.dma_start_transpose(ett[:, :], ech[:, p * 2 * T:(p + 1) * 2 * T])
                etts.append(ett)

            # gold emission: fused (feats * onehot) with free-dim accumulate
            hch = mpool.tile([BC, CHUNK, T], bf16d)
            nc.sync.dma_start(hch[:, :, :], hmask_d[:, t * CHUNK:(t + 1) * CHUNK, :])
            sc = scpool.tile([BC, CHUNK * T], f32)
            nc.vector.scalar_tensor_tensor(
                sc[:, :], fch[:, :, :].rearrange("p a b -> p (a b)"), 1.0,
                hch[:, :, :].rearrange("p a b -> p (a b)"),
                op0=ALU.mult, op1=ALU.mult,
                accum_out=parts[:, t:t + 1])

            # chain steps
            pt = None
            for sl in range(CHUNK):
                pt = pspool.tile([T + 1, BC], f32)
                nc.tensor.matmul(pt[:, :], pa_s[:, :], ucur[:, :], start=True, stop=True)
                unext = upool.tile([T, BC], bf16d)
                ett = etts[sl // 2]
                h = (sl % 2) * T
                nc.vector.tensor_tensor(unext[:, :], pt[0:T, :], ett[h:h + T, :], ALU.mult)
                ucur = unext

            # renorm controller from the last step's column sums
            if t + LAG < NCH:
                eint = scpool.tile([1, BC], i32)
                nc.vector.tensor_scalar(eint[:, :], pt[T:T + 1, :].bitcast(i32),
                                        23, None, op0=ALU.logical_shift_right)
                lam2 = scpool.tile([1, BC], f32)
                nc.vector.tensor_scalar(lam2[:, :], eint[:, :],
                                        127, -LN2 / CHUNK,
                                        op0=ALU.subtract, op1=ALU.mult)
                brow = bhpool.tile([1, BC], f32)
                nc.vector.tensor_sub(brow[:, :], lam2[:, :], bprev[:, :])
                bprev = brow
                gnew = bhpool.tile([1, BC], f32)
                nc.vector.scalar_tensor_tensor(
                    gnew[:, :], brow[:, :], float(CHUNK),
                    grow[:, :], op0=ALU.mult, op1=ALU.add)
                grow = gnew
                pbt = ps2pool.tile([BC, 1], f32)
                nc.tensor.transpose(pbt[:, :], brow[:, :], id1[:, :])
                bh = bhpool.tile([BC, 1], f32)
                nc.vector.tensor_copy(bh[:, :], pbt[:, :])
                biases.append(bh[:, :])

        # ---- finalization ----
        ptf = ps2pool.tile([1, BC], f32)
        nc.tensor.matmul(ptf[:, :], pfin_s[:, :], ucur[:, :], start=True, stop=True)

        lamf = scpool.tile([1, BC], f32)
        nc.scalar.activation(lamf[:, :], ptf[:, :], AF.Ln)
        fwdr = scpool.tile([1, BC], f32)
        nc.vector.tensor_sub(fwdr[:, :], lamf[:, :], grow[:, :])
        pfw = ps2pool.tile([BC, 1], f32)
        nc.tensor.transpose(pfw[:, :], fwdr[:, :], id1[:, :])

        # gold assembly
        emitsum = scpool.tile([BC, 1], f32)
        nc.vector.tensor_reduce(emitsum[:, :], parts[:, :], axis=AXX, op=ALU.add)
        sc16 = scpool.tile([BC, 16], f32)
        goldtr = scpool.tile([BC, 1], f32)
        nc.vector.scalar_tensor_tensor(
            sc16[:, :], rt16[:, :], 1.0, m16_s[:, :],
            op0=ALU.mult, op1=ALU.mult, accum_out=goldtr[:, :])

        sc0 = scpool.tile([BC, T], f32)
        s0col = scpool.tile([BC, 1], f32)
        nc.vector.scalar_tensor_tensor(
            sc0[:, :], startw_s[:, :], 1.0, h0_s[:, :],
            op0=ALU.mult, op1=ALU.mult, accum_out=s0col[:, :])
        scL = scpool.tile([BC, T], f32)
        sLcol = scpool.tile([BC, 1], f32)
        nc.vector.scalar_tensor_tensor(
            scL[:, :], startw_s[:, :], 1.0, hL_s[:, :],
            op0=ALU.mult, op1=ALU.mult, accum_out=sLcol[:, :])

        g1 = scpool.tile([BC, 1], f32)
        nc.vector.tensor_add(g1[:, :], s0col[:, :], sLcol[:, :])
        g2 = scpool.tile([BC, 1], f32)
        nc.vector.tensor_add(g2[:, :], g1[:, :], emitsum[:, :])
        g3 = scpool.tile([BC, 1], f32)
        nc.vector.tensor_add(g3[:, :], g2[:, :], goldtr[:, :])

        l0 = scpool.tile([BC, 1], f32)
        nc.vector.tensor_sub(l0[:, :], pfw[:, :], g3[:, :])
        lout = scpool.tile([BC, 1], f32)
        nc.vector.tensor_scalar(lout[:, :], l0[:, :], -S * LN_SCALE, None, op0=ALU.add)
        nc.sync.dma_start(out_d[:, :], lout[:, :])

    nc.compile()
    return nc


def _host_constants(transitions, start_tag, tags):
    """Small host-side constant tensors (index plumbing + exp of the tiny
    transition matrix); tags comes in as [B, S] int."""
    pa = np.zeros((T, T + 1), dtype=np.float32)
    pa[:, :T] = np.exp(transitions.T + LN_SCALE)
    pa[:, T] = 1.0
    pa = pa.astype(bf16)
    pfin = np.exp(transitions[T - 1, :]).astype(bf16).reshape(T, 1)
    u0 = np.tile(np.exp(start_tag).astype(np.float32)[:, None], (1, BC)).astype(bf16)
    startw = np.tile(start_tag.astype(np.float32)[None, :], (BC, 1))
    transrep = np.tile(transitions.astype(np.float32).reshape(1, T * T), (BC, 1))
    m16 = np.zeros((BC, 16), dtype=bf16)
    for p in range(BC):
        m16[p, p % 16] = 1

    # one-hot of the gold tags, bf16 (streamed next to feats for the
    # emission-score multiply-accumulate)
    tags_i = tags.astype(np.int64)
    hmask = (tags_i[:, :, None] == np.arange(T)[None, None, :]).astype(bf16)

    # wrapped pair indices for the indirect_copy gathers: instr i,
    # 16-partition group g handles example b = g*16 + i; unwrapped order is
    # (c*16 + p).
    pairs = np.zeros((B, 512), dtype=np.uint16)
    pairs[:, :511] = (tags_i[:, :511] * T + tags_i[:, 1:512]).astype(np.uint16)
    gi, pi, ci = np.meshgrid(np.arange(8), np.arange(16), np.arange(32),
                             indexing="ij")
    pairsw = np.zeros((NC, BC, 16 * 32), dtype=np.uint16)
    for c in range(NC):
        pc = pairs[c * BC:(c + 1) * BC]
        for i in range(16):
            b = gi * 16 + i
            s = ci * 16 + pi
            pairsw[c, (16 * gi + pi).reshape(-1), (i * 32 + ci).reshape(-1)] =                 pc[b.reshape(-1), s.reshape(-1)]
    return pa, pfin, u0, startw, transrep, m16, pairsw, hmask


_NC_CACHE = {}


def _get_program():
    if "nc" not in _NC_CACHE:
        _NC_CACHE["nc"] = _build_program()
    return _NC_CACHE["nc"]


def kernel(feats, transitions, start_tag, tags, mask_x, len_seq):
    feats = np.asarray(feats, dtype=np.float32)
    transitions = np.asarray(transitions, dtype=np.float32)
    start_tag = np.asarray(start_tag, dtype=np.float32)
    tags_np = np.asarray(tags)
    out_dtype = np.float32

    pa, pfin, u0, startw, transrep, m16, pairsw, hmask = \
        _host_constants(transitions, start_tag, tags_np)

    in_maps = []
    for c in range(NC):
        sl = slice(c * BC, (c + 1) * BC)
        in_maps.append({
            "feats": np.ascontiguousarray(feats[sl]),
            "hmask": np.ascontiguousarray(hmask[sl]),
            "u0": u0, "pa": pa, "pfin": pfin, "startw": startw,
            "transrep": transrep, "pairsw": pairsw[c], "m16": m16,
        })

    nc = _get_program()
    res = run_bass_kernel_spmd(nc, in_maps, list(range(NC)))
    out = np.concatenate([res.results[i]["out"][:, 0] for i in range(NC)])
    return out.astype(out_dtype)



# revision 10
# speedup vs baseline: 1.8757x; 1.8757x over previous
"""CRF log-loss kernel for TRN2, data-parallel over batch on 8 NeuronCores.

Forward algorithm restructured for latency hiding:
  * The S=512-step sequence is split into NSEG=4 segments of 128 steps.
    Segment products are joined with rank-1 cross-approximation seams
    (error ~ (lambda2/lambda1)^128, far below tolerance):
        Z ~= (r3.c2)(g2.c1)(g1.c0) / (sum c2)(sum c1)
    where c_q are forward chains over segments 0..2 and g/r are backward
    chains over segments 1..3, all recursions of the same per-step cost and
    mutually independent -> 128 sequential rounds instead of 512.
  * Each round runs Q=3 paired chains: one 128x128 matmul against a
    resident block-diagonal stationary (fwd transition matrix in the top-left
    64x64, transposed one in the bottom-right) + one DVE multiply with the
    exp'd emissions.  Forward chain state lives in partitions 0-63, backward
    in 64-127.
  * exp(feats) runs on the scalar engine with host-calibrated per-chunk bias
    constants (no device-side renorm feedback), reading host-pre-transposed
    bf16 feats -> no DMA transposes, no serialization with the chain.
  * Gold score (emissions + transitions + start terms) is one fused gpsimd
    gather per 16-example rotation from a combined bf16 table, reduced with
    activation-accumulate on the scalar engine.
"""
import numpy as np
import ml_dtypes
from contextlib import ExitStack

import concourse.bass as bass
import concourse.bacc as bacc
import concourse.tile as tile
import concourse.mybir as mybir
from concourse.bass_utils import run_bass_kernel_spmd

bf16 = ml_dtypes.bfloat16
f32 = mybir.dt.float32
bf16d = mybir.dt.bfloat16
u16 = mybir.dt.uint16

B, S, T = 1024, 512, 64
NC = 8
BC = B // NC            # 128 examples per core
NSEG = 4
L = S // NSEG           # 128 rounds
Q = NSEG - 1            # 3 paired fwd/bwd chains
GRP = 8                 # rounds per exp group (= beta chunk size in steps)
NGRP = L // GRP         # 16 groups per chain
CHUNK = 8               # beta granularity in steps
NCH = S // CHUNK        # 64 chunks
NIDX = 1024             # gather indices per example: 511 pairs + pad + 512 emis
GVAL = 512              # gathered values per indirect_copy instruction
GT_TRANS = 0            # gather-table section offsets
GT_FEAT = T * T
GT_ZERO = GT_FEAT + S * T   # guaranteed-zero pad cell
GT_SIZE = GT_ZERO + 16      # 36880 (16-elem padded)

AF = mybir.ActivationFunctionType
ALU = mybir.AluOpType


def _build_program():
    nc = bacc.Bacc("TRN2", target_bir_lowering=False, debug=False, num_devices=NC)

    ft_d = nc.dram_tensor("ftp", [128, Q * L * BC], bf16d, kind="ExternalInput")
    gtab_d = nc.dram_tensor("gtab", [BC, GT_SIZE], bf16d, kind="ExternalInput")
    gidx_d = nc.dram_tensor("gidx", [BC, NIDX], u16, kind="ExternalInput")
    bd_d = nc.dram_tensor("bd", [128, 128], bf16d, kind="ExternalInput")
    bdf_d = nc.dram_tensor("bdf", [128, T], bf16d, kind="ExternalInput")
    init_d = nc.dram_tensor("init", [128, Q * 128], bf16d, kind="ExternalInput")
    biasall_d = nc.dram_tensor("biasall", [128, Q * NGRP], f32, kind="ExternalInput")
    bias1_d = nc.dram_tensor("bias1", [BC, 1], f32, kind="ExternalInput")
    m16_d = nc.dram_tensor("m16", [BC, 32], bf16d, kind="ExternalInput")
    out_d = nc.dram_tensor("out", [BC, 1], f32, kind="ExternalOutput")

    with tile.TileContext(nc) as tc, ExitStack() as ctx:
        cpool = ctx.enter_context(tc.tile_pool(name="const", bufs=1))
        gpool = ctx.enter_context(tc.tile_pool(name="gout", bufs=32))
        scpool = ctx.enter_context(tc.tile_pool(name="scratch", bufs=2))
        ftpools = [ctx.enter_context(tc.tile_pool(name=f"ft{q}", bufs=2))
                   for q in range(Q)]
        etpools = [ctx.enter_context(tc.tile_pool(name=f"et{q}", bufs=2))
                   for q in range(Q)]
        stpools = [ctx.enter_context(tc.tile_pool(name=f"st{q}", bufs=2))
                   for q in range(Q)]
        pspools = [ctx.enter_context(tc.tile_pool(name=f"ps{q}", bufs=2, space="PSUM"))
                   for q in range(Q)]
        psfpool = ctx.enter_context(tc.tile_pool(name="psf", bufs=1, space="PSUM"))
        psdpool = ctx.enter_context(tc.tile_pool(name="psd", bufs=1, space="PSUM"))

        # ---- big gather table on the ACT DGE queue (runs concurrently with
        # the small const DMAs on the sync queue) ----
        gtab_s = cpool.tile([BC, GT_SIZE], bf16d)
        nc.scalar.dma_start(gtab_s[:, :], gtab_d[:, :])

        # ---- small constants ----
        bd_s = cpool.tile([128, 128], bf16d)
        nc.sync.dma_start(bd_s[:, :], bd_d[:, :])
        bdf_s = cpool.tile([128, T], bf16d)
        nc.sync.dma_start(bdf_s[:, :], bdf_d[:, :])
        init_s = cpool.tile([128, Q * 128], bf16d)
        nc.sync.dma_start(init_s[:, :], init_d[:, :])
        biasall_s = cpool.tile([128, Q * NGRP], f32)
        nc.sync.dma_start(biasall_s[:, :], biasall_d[:, :])
        bias1_s = cpool.tile([BC, 1], f32)
        nc.sync.dma_start(bias1_s[:, :], bias1_d[:, :])
        m16_s = cpool.tile([BC, 32], bf16d)
        nc.sync.dma_start(m16_s[:, :], m16_d[:, :])
        gidx_s = cpool.tile([BC, NIDX], u16)
        nc.sync.dma_start(gidx_s[:, :], gidx_d[:, :])

        ones64 = cpool.tile([T, 1], bf16d)
        nc.vector.memset(ones64[:, :], 1.0)
        rt16 = cpool.tile([BC, 32], f32)

        # ---- gold gathers (gpsimd; priming: one sync wait per pool instr) ----
        pr1 = scpool.tile([BC, 1], bf16d)
        nc.gpsimd.tensor_copy(pr1[:, :], gtab_s[:, 0:1])
        pr2 = scpool.tile([BC, 1], u16)
        nc.gpsimd.tensor_copy(pr2[:, :], gidx_s[:, 0:1])
        gouts = []
        for i in range(32):
            gout = gpool.tile([BC, GVAL], bf16d)
            nc.gpsimd.indirect_copy(
                gout[:, :].unsqueeze(-1),
                gtab_s[:, :],
                gidx_s[:, i * 32:(i + 1) * 32],
                i_know_ap_gather_is_preferred=True,
            )
            gouts.append(gout)

        # ---- main rounds ----
        etts = [None] * Q
        stprev = [None] * Q
        reduces_done = 0
        scr = cpool.tile([BC, GVAL], bf16d)

        def emit_reduce(i):
            nc.scalar.activation(scr[:, :], gouts[i][:, :], AF.Copy,
                                 accum_out=rt16[:, i:i + 1])

        for r in range(L):
            if r % GRP == 0:
                g = r // GRP
                for q in range(Q):
                    ft = ftpools[q].tile([128, GRP * BC], bf16d)
                    base = q * (L * BC) + g * (GRP * BC)
                    nc.sync.dma_start(ft[:, :], ft_d[:, base:base + GRP * BC])
                    ett = etpools[q].tile([128, GRP * BC], bf16d)
                    nc.scalar.activation(ett[:, :], ft[:, :], AF.Exp,
                                         bias=biasall_s[:, q * NGRP + g:q * NGRP + g + 1],
                                         scale=1.0)
                    etts[q] = ett
                # interleave gold reduces on ACT once the gathers are surely
                # done (they finish ~2.6us apart behind a ~26us table DMA)
                if g >= 6:
                    for _ in range(3):
                        if reduces_done < 32:
                            emit_reduce(reduces_done)
                            reduces_done += 1
            sl = (r % GRP) * BC
            for q in range(Q):
                st = stpools[q].tile([128, BC], bf16d)
                if r == 0:
                    nc.vector.tensor_tensor(
                        st[:, :], init_s[:, q * 128:(q + 1) * 128],
                        etts[q][:, sl:sl + BC], ALU.mult)
                else:
                    ps = pspools[q].tile([128, 512], f32)
                    nc.tensor.matmul(ps[:, 0:BC], bd_s[:, :], stprev[q][:, :],
                                     start=True, stop=True)
                    nc.vector.tensor_tensor(
                        st[:, :], ps[:, 0:BC], etts[q][:, sl:sl + BC], ALU.mult)
                stprev[q] = st

        # ---- finals: g_q = T^T h_q placed in partitions 0-63, then the
        # per-example seam products z_q = g_q * c_q ----
        zs = []
        for q in range(Q):
            psf = psfpool.tile([128, 512], f32)
            nc.tensor.matmul(psf[0:T, 0:BC], bdf_s[:, :], stprev[q][:, :],
                             start=True, stop=True)
            z = scpool.tile([T, BC], bf16d, name=f"z{q}")
            nc.vector.tensor_tensor(z[:, :], psf[0:T, 0:BC],
                                    stprev[q][0:T, :], ALU.mult)
            zs.append(z)

        while reduces_done < 32:
            emit_reduce(reduces_done)
            reduces_done += 1

        # ---- seam dots: column sums via matmul against ones ----
        psd = psdpool.tile([128, 512], f32)
        dot_srcs = [zs[2][:, :], zs[1][:, :], zs[0][:, :],
                    stprev[2][0:T, :], stprev[1][0:T, :]]
        for j, src in enumerate(dot_srcs):
            nc.tensor.matmul(psd[:, j:j + 1], src, ones64[:, :],
                             start=True, stop=True)
        lns = scpool.tile([128, 5], f32)
        nc.scalar.activation(lns[:, :], psd[:, 0:5], AF.Ln)

        # ---- gold combine ----
        sc16 = scpool.tile([BC, 32], f32)
        goldcol = scpool.tile([BC, 1], f32)
        nc.vector.scalar_tensor_tensor(
            sc16[:, :], rt16[:, :], 1.0, m16_s[:, :],
            op0=ALU.mult, op1=ALU.mult, accum_out=goldcol[:, :])

        # ---- assemble: logZ = lnA+lnB+lnC-lnD-lnE + bias1 ; out = logZ-gold
        t1 = scpool.tile([BC, 1], f32)
        nc.vector.tensor_add(t1[:, :], lns[:, 0:1], lns[:, 1:2])
        t2 = scpool.tile([BC, 1], f32)
        nc.vector.tensor_add(t2[:, :], t1[:, :], lns[:, 2:3])
        t3 = scpool.tile([BC, 1], f32)
        nc.vector.tensor_sub(t3[:, :], t2[:, :], lns[:, 3:4])
        t4 = scpool.tile([BC, 1], f32)
        nc.vector.tensor_sub(t4[:, :], t3[:, :], lns[:, 4:5])
        t5 = scpool.tile([BC, 1], f32)
        nc.vector.tensor_add(t5[:, :], t4[:, :], bias1_s[:, :])
        lout = scpool.tile([BC, 1], f32)
        nc.vector.tensor_sub(lout[:, :], t5[:, :], goldcol[:, :])
        nc.sync.dma_start(out_d[:, :], lout[:, :])

    nc.compile()
    return nc


def _calibrate_beta(feats, transitions, start_tag, n_cal=8):
    """Per-chunk mean log-growth of the forward recursion, from a few
    examples, used as compile-free device bias constants."""
    Tm = np.exp(transitions.astype(np.float64))
    idx = np.linspace(0, B - 1, n_cal).astype(np.int64)
    u = np.tile(np.exp(start_tag.astype(np.float64))[None, :], (n_cal, 1))
    growth = np.zeros((n_cal, S))
    f = feats[idx].astype(np.float64)
    for s in range(S):
        u2 = np.exp(f[:, s, :]) * (u @ Tm.T)
        z = u2.sum(axis=1)
        growth[:, s] = np.log(z)
        u = u2 / z[:, None]
    g = growth.mean(axis=0)
    beta = -g.reshape(NCH, CHUNK).mean(axis=1)  # [NCH] per chunk
    return np.repeat(beta, CHUNK)               # [S] per step


def _host_prep(feats, transitions, start_tag, tags):
    """Shared (cross-core) constants + per-core tensors."""
    trans64 = transitions.astype(np.float64)
    Tm = np.exp(trans64)                       # T[j,k] = exp(trans[j,k])
    beta = _calibrate_beta(feats, transitions, start_tag)

    # block-diag stationary: BD[k,j]=T[j,k] (fwd), BD[64+k,64+j]=T[k,j] (bwd)
    bd = np.zeros((128, 128), dtype=np.float64)
    bd[:T, :T] = Tm.T
    bd[T:, T:] = Tm
    bd = bd.astype(bf16)
    # final bwd matmul: out[j] = sum_k T[k,j] h[k], j in partitions 0-63
    bdf = np.zeros((128, T), dtype=np.float64)
    bdf[T:, :] = Tm
    bdf = bdf.astype(bf16)

    # init tiles: top = (T @ u_start) replicated, bottom = p0 replicated
    u0 = np.exp(start_tag.astype(np.float64))
    pfin = Tm[T - 1, :]                        # exp(trans[63, :])
    init = np.zeros((128, Q * 128), dtype=np.float64)
    for q in range(Q):
        top = Tm @ (u0 if q == 0 else np.ones(T))
        bot = pfin if q == Q - 1 else np.ones(T)
        init[:T, q * 128:(q + 1) * 128] = top[:, None]
        init[T:, q * 128:(q + 1) * 128] = bot[:, None]
    init = init.astype(bf16)

    # per-(q, group) exp bias: top rows get the fwd chunk's beta, bottom rows
    # the bwd chunk's
    biasall = np.zeros((128, Q * NGRP), dtype=np.float32)
    for q in range(Q):
        for g in range(NGRP):
            cf = q * NGRP + g                  # fwd chunk index
            cb = (q + 2) * NGRP - 1 - g        # bwd chunk index
            biasall[:T, q * NGRP + g] = beta[cf * CHUNK]
            biasall[T:, q * NGRP + g] = beta[cb * CHUNK]
    bias1 = np.full((BC, 1), -beta.sum(), dtype=np.float32)

    m16 = np.zeros((BC, 32), dtype=bf16)
    for p in range(BC):
        m16[p, p % 16] = 1
        m16[p, 16 + p % 16] = 1

    # transposed emissions, round-major: ftp[p, q, r, b]
    #   p in 0..63  (tag): feats[b, q*128+r, tag]
    #   p in 64..127     : feats[b, (q+2)*128-1-r, tag]
    fs = np.ascontiguousarray(feats.transpose(1, 2, 0))  # [S, T, B]
    fwd = fs[:Q * L].reshape(Q, L, T, B).transpose(2, 0, 1, 3)
    bwd = fs[L:].reshape(Q, L, T, B)[:, ::-1].transpose(2, 0, 1, 3)
    ftp_full = np.concatenate([fwd, bwd], axis=0).astype(bf16)  # [128,Q,L,B]

    # gather table sections + wrapped indices.  start_tag terms are folded
    # into the s=0 and s=S-1 emission entries; one pad index per example
    # points at a guaranteed-zero cell.
    tags_i = tags.astype(np.int64)
    trans_flat = transitions.astype(bf16).reshape(1, T * T)
    femis = feats.copy()
    femis[:, 0, :] += start_tag[None, :]
    femis[:, S - 1, :] += start_tag[None, :]
    idxlist = np.zeros((B, NIDX), dtype=np.uint16)
    idxlist[:, :S - 1] = (tags_i[:, :S - 1] * T + tags_i[:, 1:]).astype(np.uint16)
    idxlist[:, S - 1] = GT_ZERO
    idxlist[:, S:2 * S] = (GT_FEAT + np.arange(S)[None, :] * T
                           + tags_i).astype(np.uint16)

    gi, pi, ci = np.meshgrid(np.arange(8), np.arange(16), np.arange(32),
                             indexing="ij")
    shared = dict(bd=bd, bdf=bdf, init=init, biasall=biasall, bias1=bias1,
                  m16=m16)
    in_maps = []
    for c in range(NC):
        sl = slice(c * BC, (c + 1) * BC)
        ftp = np.ascontiguousarray(ftp_full[:, :, :, sl]).reshape(128, Q * L * BC)
        gtab = np.zeros((BC, GT_SIZE), dtype=bf16)
        gtab[:, :GT_FEAT] = trans_flat
        gtab[:, GT_FEAT:GT_ZERO] = femis[sl].astype(bf16).reshape(BC, S * T)
        il = idxlist[sl]
        gidx = np.zeros((BC, NIDX), dtype=np.uint16)
        for h in range(2):
            for i in range(16):
                b = gi * 16 + i
                pos = h * GVAL + ci * 16 + pi
                gidx[(16 * gi + pi).reshape(-1),
                     (h * GVAL + i * 32 + ci).reshape(-1)] = \
                    il[b.reshape(-1), pos.reshape(-1)]
        im = {"ftp": ftp, "gtab": gtab, "gidx": gidx}
        im.update(shared)
        in_maps.append(im)
    return in_maps


_NC_CACHE = {}


def _get_program():
    if "nc" not in _NC_CACHE:
        _NC_CACHE["nc"] = _build_program()
    return _NC_CACHE["nc"]


def kernel(feats, transitions, start_tag, tags, mask_x, len_seq):
    feats = np.asarray(feats, dtype=np.float32)
    transitions = np.asarray(transitions, dtype=np.float32)
    start_tag = np.asarray(start_tag, dtype=np.float32)
    tags_np = np.asarray(tags)

    in_maps = _host_prep(feats, transitions, start_tag, tags_np)
    nc = _get_program()
    res = run_bass_kernel_spmd(nc, in_maps, list(range(NC)))
    out = np.concatenate([res.results[i]["out"][:, 0] for i in range(NC)])
    return out.astype(np.float32)


# revision 14
# speedup vs baseline: 6.8982x; 3.6777x over previous
"""CRF log-loss kernel for TRN2, data-parallel over batch on 8 NeuronCores.

Forward algorithm restructured for latency hiding:
  * The S=512-step sequence is split into NSEG=4 segments of 128 steps.
    Segment products are joined with rank-1 cross-approximation seams
    (error ~ (lambda2/lambda1)^128, far below tolerance):
        Z ~= (r3.c2)(g2.c1)(g1.c0) / (sum c2)(sum c1)
    where c_q are forward chains over segments 0..2 and g/r are backward
    chains over segments 1..3, all recursions of the same per-step cost and
    mutually independent -> 128 sequential rounds instead of 512.
  * Each round runs Q=3 paired chains: one 128x128 matmul against a
    resident block-diagonal stationary (fwd transition matrix in the top-left
    64x64, transposed one in the bottom-right) + one DVE multiply with the
    exp'd emissions.  Forward chain state lives in partitions 0-63, backward
    in 64-127.
  * exp(feats) runs on the scalar engine with host-calibrated per-chunk bias
    constants (no device-side renorm feedback), reading host-pre-transposed
    bf16 feats -> no DMA transposes, no serialization with the chain.
  * Gold score: transition+start terms as <per-example pair-count vector,
    [trans; start; start]> via 33 PSUM-accumulated matmuls on otherwise-idle
    PE cycles; emission term as one-hot x feats multiply-accumulate on the
    otherwise-idle gpsimd engine.
"""
import numpy as np
import ml_dtypes
from contextlib import ExitStack

import concourse.bass as bass
import concourse.bacc as bacc
import concourse.tile as tile
import concourse.mybir as mybir
from concourse.bass_utils import run_bass_kernel_spmd

bf16 = ml_dtypes.bfloat16
f32 = mybir.dt.float32
f16 = mybir.dt.float16
bf16d = mybir.dt.bfloat16
u16 = mybir.dt.uint16

B, S, T = 1024, 512, 64
NC = 8
BC = B // NC            # 128 examples per core
NSEG = 4
L = S // NSEG           # 128 rounds
Q = NSEG - 1            # 3 paired fwd/bwd chains
GRP = 8                 # rounds per exp group (= beta chunk size in steps)
NGRP = L // GRP         # 16 groups per chain
CHUNK = 8               # beta granularity in steps
NCH = S // CHUNK        # 64 chunks
NCV = T * T + 2 * T     # 4224: transition pairs + start-tag(first) + start-tag(last)
NCK = NCV // 128        # 33 contraction chunks for the gold count-matmul
NWIN = 16               # emission windows
WSZ = S * T // NWIN     # 2048 elements per window

AF = mybir.ActivationFunctionType
ALU = mybir.AluOpType


def _build_program():
    nc = bacc.Bacc("TRN2", target_bir_lowering=False, debug=False, num_devices=NC)

    ft_d = nc.dram_tensor("ftp", [128, Q * L * BC], bf16d, kind="ExternalInput")
    cnt_d = nc.dram_tensor("cnt", [128, NCK * BC], f16, kind="ExternalInput")
    trv_d = nc.dram_tensor("trv", [128, NCK], f16, kind="ExternalInput")
    hm_d = nc.dram_tensor("hm", [BC, S * T], bf16d, kind="ExternalInput")
    ff_d = nc.dram_tensor("ff", [BC, S * T], bf16d, kind="ExternalInput")
    bd_d = nc.dram_tensor("bd", [128, 128], bf16d, kind="ExternalInput")
    bdf_d = nc.dram_tensor("bdf", [128, T], bf16d, kind="ExternalInput")
    init_d = nc.dram_tensor("init", [128, Q * 128], bf16d, kind="ExternalInput")
    biasall_d = nc.dram_tensor("biasall", [128, Q * NGRP], f32, kind="ExternalInput")
    bias1_d = nc.dram_tensor("bias1", [BC, 1], f32, kind="ExternalInput")
    out_d = nc.dram_tensor("out", [BC, 1], f32, kind="ExternalOutput")

    with tile.TileContext(nc) as tc, ExitStack() as ctx:
        cpool = ctx.enter_context(tc.tile_pool(name="const", bufs=1))
        hmpool = ctx.enter_context(tc.tile_pool(name="hmw", bufs=2))
        ffpool = ctx.enter_context(tc.tile_pool(name="ffw", bufs=2))
        prpool = ctx.enter_context(tc.tile_pool(name="prw", bufs=2))
        scpool = ctx.enter_context(tc.tile_pool(name="scratch", bufs=2))
        ftpools = [ctx.enter_context(tc.tile_pool(name=f"ft{q}", bufs=2))
                   for q in range(Q)]
        etpools = [ctx.enter_context(tc.tile_pool(name=f"et{q}", bufs=2))
                   for q in range(Q)]
        stpools = [ctx.enter_context(tc.tile_pool(name=f"st{q}", bufs=2))
                   for q in range(Q)]
        pspools = [ctx.enter_context(tc.tile_pool(name=f"ps{q}", bufs=2, space="PSUM"))
                   for q in range(Q)]
        psfpool = ctx.enter_context(tc.tile_pool(name="psf", bufs=1, space="PSUM"))
        psdpool = ctx.enter_context(tc.tile_pool(name="psd", bufs=1, space="PSUM"))

        # ---- small constants ----
        bd_s = cpool.tile([128, 128], bf16d)
        nc.sync.dma_start(bd_s[:, :], bd_d[:, :])
        bdf_s = cpool.tile([128, T], bf16d)
        nc.sync.dma_start(bdf_s[:, :], bdf_d[:, :])
        init_s = cpool.tile([128, Q * 128], bf16d)
        nc.sync.dma_start(init_s[:, :], init_d[:, :])
        biasall_s = cpool.tile([128, Q * NGRP], f32)
        nc.sync.dma_start(biasall_s[:, :], biasall_d[:, :])
        bias1_s = cpool.tile([BC, 1], f32)
        nc.sync.dma_start(bias1_s[:, :], bias1_d[:, :])
        cnt_s = cpool.tile([128, NCK * BC], f16)
        nc.sync.dma_start(cnt_s[:, :], cnt_d[:, :])
        trv_s = cpool.tile([128, NCK], f16)
        nc.sync.dma_start(trv_s[:, :], trv_d[:, :])

        ones64 = cpool.tile([T, 1], bf16d)
        nc.vector.memset(ones64[:, :], 1.0)
        emis4 = cpool.tile([BC, NWIN], f32)
        ascr = cpool.tile([BC, WSZ], bf16d)

        # ---- main rounds ----
        etts = [None] * Q
        stprev = [None] * Q
        win_done = 0

        def emit_window(w):
            hm = hmpool.tile([BC, WSZ], bf16d)
            nc.sync.dma_start(hm[:, :], hm_d[:, w * WSZ:(w + 1) * WSZ])
            ffw = ffpool.tile([BC, WSZ], bf16d)
            nc.sync.dma_start(ffw[:, :], ff_d[:, w * WSZ:(w + 1) * WSZ])
            prod = prpool.tile([BC, WSZ], bf16d)
            nc.gpsimd.tensor_tensor(prod[:, :], hm[:, :], ffw[:, :], ALU.mult)
            nc.scalar.activation(ascr[:, :], prod[:, :], AF.Copy,
                                 accum_out=emis4[:, w:w + 1])

        for r in range(L):
            if r % GRP == 0:
                g = r // GRP
                for q in range(Q):
                    ft = ftpools[q].tile([128, GRP * BC], bf16d)
                    base = q * (L * BC) + g * (GRP * BC)
                    nc.sync.dma_start(ft[:, :], ft_d[:, base:base + GRP * BC])
                    ett = etpools[q].tile([128, GRP * BC], bf16d)
                    nc.scalar.activation(ett[:, :], ft[:, :], AF.Exp,
                                         bias=biasall_s[:, q * NGRP + g:q * NGRP + g + 1],
                                         scale=1.0)
                    etts[q] = ett
                # spread emission windows across the run (pool + scalar)
                if win_done < NWIN:
                    emit_window(win_done)
                    win_done += 1
            sl = (r % GRP) * BC
            for q in range(Q):
                st = stpools[q].tile([128, BC], bf16d)
                if r == 0:
                    nc.vector.tensor_tensor(
                        st[:, :], init_s[:, q * 128:(q + 1) * 128],
                        etts[q][:, sl:sl + BC], ALU.mult)
                else:
                    ps = pspools[q].tile([128, 512], f32)
                    nc.tensor.matmul(ps[:, 0:BC], bd_s[:, :], stprev[q][:, :],
                                     start=True, stop=True)
                    nc.vector.tensor_tensor(
                        st[:, :], ps[:, 0:BC], etts[q][:, sl:sl + BC], ALU.mult)
                stprev[q] = st

        # ---- finals: g_q = T^T h_q placed in partitions 0-63, then the
        # per-example seam products z_q = g_q * c_q ----
        zs = []
        for q in range(Q):
            psf = psfpool.tile([128, 512], f32)
            nc.tensor.matmul(psf[0:T, 0:BC], bdf_s[:, :], stprev[q][:, :],
                             start=True, stop=True)
            z = scpool.tile([T, BC], bf16d, name=f"z{q}")
            nc.vector.tensor_tensor(z[:, :], psf[0:T, 0:BC],
                                    stprev[q][0:T, :], ALU.mult)
            zs.append(z)

        # ---- seam dots: column sums via matmul against ones ----
        psd = psdpool.tile([128, 512], f32)
        dot_srcs = [zs[2][:, :], zs[1][:, :], zs[0][:, :],
                    stprev[2][0:T, :], stprev[1][0:T, :]]
        for j, src in enumerate(dot_srcs):
            nc.tensor.matmul(psd[:, j:j + 1], src, ones64[:, :],
                             start=True, stop=True)
        # gold transitions+start: <counts, [trans; start; start]> via
        # PSUM-accumulated matmuls
        for j in range(NCK):
            nc.tensor.matmul(psd[:, 8:9], cnt_s[:, j * BC:(j + 1) * BC],
                             trv_s[:, j:j + 1], start=(j == 0),
                             stop=(j == NCK - 1))
        lns = scpool.tile([128, 5], f32)
        nc.scalar.activation(lns[:, :], psd[:, 0:5], AF.Ln)

        # ---- gold combine ----
        emisum = scpool.tile([BC, 1], f32)
        nc.vector.tensor_reduce(emisum[:, :], emis4[:, :],
                                axis=mybir.AxisListType.X, op=ALU.add)
        goldcol = scpool.tile([BC, 1], f32)
        nc.vector.tensor_add(goldcol[:, :], emisum[:, :], psd[:, 8:9])

        # ---- assemble: logZ = lnA+lnB+lnC-lnD-lnE + bias1 ; out = logZ-gold
        t1 = scpool.tile([BC, 1], f32)
        nc.vector.tensor_add(t1[:, :], lns[:, 0:1], lns[:, 1:2])
        t2 = scpool.tile([BC, 1], f32)
        nc.vector.tensor_add(t2[:, :], t1[:, :], lns[:, 2:3])
        t3 = scpool.tile([BC, 1], f32)
        nc.vector.tensor_sub(t3[:, :], t2[:, :], lns[:, 3:4])
        t4 = scpool.tile([BC, 1], f32)
        nc.vector.tensor_sub(t4[:, :], t3[:, :], lns[:, 4:5])
        t5 = scpool.tile([BC, 1], f32)
        nc.vector.tensor_add(t5[:, :], t4[:, :], bias1_s[:, :])
        lout = scpool.tile([BC, 1], f32)
        nc.vector.tensor_sub(lout[:, :], t5[:, :], goldcol[:, :])
        nc.sync.dma_start(out_d[:, :], lout[:, :])

    nc.compile()
    return nc


def _calibrate_beta(feats, transitions, start_tag, n_cal=8):
    """Per-chunk mean log-growth of the forward recursion, from a few
    examples, used as compile-free device bias constants."""
    Tm = np.exp(transitions.astype(np.float64))
    idx = np.linspace(0, B - 1, n_cal).astype(np.int64)
    u = np.tile(np.exp(start_tag.astype(np.float64))[None, :], (n_cal, 1))
    growth = np.zeros((n_cal, S))
    f = feats[idx].astype(np.float64)
    for s in range(S):
        u2 = np.exp(f[:, s, :]) * (u @ Tm.T)
        z = u2.sum(axis=1)
        growth[:, s] = np.log(z)
        u = u2 / z[:, None]
    g = growth.mean(axis=0)
    beta = -g.reshape(NCH, CHUNK).mean(axis=1)  # [NCH] per chunk
    return np.repeat(beta, CHUNK)               # [S] per step


def _host_prep(feats, transitions, start_tag, tags):
    """Shared (cross-core) constants + per-core tensors."""
    trans64 = transitions.astype(np.float64)
    Tm = np.exp(trans64)                       # T[j,k] = exp(trans[j,k])
    beta = _calibrate_beta(feats, transitions, start_tag)

    # block-diag stationary: BD[k,j]=T[j,k] (fwd), BD[64+k,64+j]=T[k,j] (bwd)
    bd = np.zeros((128, 128), dtype=np.float64)
    bd[:T, :T] = Tm.T
    bd[T:, T:] = Tm
    bd = bd.astype(bf16)
    # final bwd matmul: out[j] = sum_k T[k,j] h[k], j in partitions 0-63
    bdf = np.zeros((128, T), dtype=np.float64)
    bdf[T:, :] = Tm
    bdf = bdf.astype(bf16)

    # init tiles: top = (T @ u_start) replicated, bottom = p0 replicated
    u0 = np.exp(start_tag.astype(np.float64))
    pfin = Tm[T - 1, :]                        # exp(trans[63, :])
    init = np.zeros((128, Q * 128), dtype=np.float64)
    for q in range(Q):
        top = Tm @ (u0 if q == 0 else np.ones(T))
        bot = pfin if q == Q - 1 else np.ones(T)
        init[:T, q * 128:(q + 1) * 128] = top[:, None]
        init[T:, q * 128:(q + 1) * 128] = bot[:, None]
    init = init.astype(bf16)

    # per-(q, group) exp bias: top rows get the fwd chunk's beta, bottom rows
    # the bwd chunk's
    biasall = np.zeros((128, Q * NGRP), dtype=np.float32)
    for q in range(Q):
        for g in range(NGRP):
            cf = q * NGRP + g                  # fwd chunk index
            cb = (q + 2) * NGRP - 1 - g        # bwd chunk index
            biasall[:T, q * NGRP + g] = beta[cf * CHUNK]
            biasall[T:, q * NGRP + g] = beta[cb * CHUNK]
    bias1 = np.full((BC, 1), -beta.sum(), dtype=np.float32)


    # transposed emissions, round-major: ftp[p, q, r, b]
    #   p in 0..63  (tag): feats[b, q*128+r, tag]
    #   p in 64..127     : feats[b, (q+2)*128-1-r, tag]
    fs = np.ascontiguousarray(feats.transpose(1, 2, 0))  # [S, T, B]
    fwd = fs[:Q * L].reshape(Q, L, T, B).transpose(2, 0, 1, 3)
    bwd = fs[L:].reshape(Q, L, T, B)[:, ::-1].transpose(2, 0, 1, 3)
    ftp_full = np.concatenate([fwd, bwd], axis=0).astype(bf16)  # [128,Q,L,B]

    # gold-score inputs: per-example counts against [trans; start; start],
    # plus one-hot emission mask
    tags_i = tags.astype(np.int64)
    vec = np.concatenate([transitions.reshape(-1), start_tag, start_tag])
    trv = vec.astype(np.float16).reshape(NCK, 128).T.copy()     # [128, NCK]
    counts = np.zeros((B, NCV), dtype=np.float16)
    bidx = np.repeat(np.arange(B), S - 1)
    pairs = (tags_i[:, :S - 1] * T + tags_i[:, 1:]).reshape(-1)
    np.add.at(counts, (bidx, pairs), 1.0)
    np.add.at(counts, (np.arange(B), T * T + tags_i[:, 0]), 1.0)
    np.add.at(counts, (np.arange(B), T * T + T + tags_i[:, S - 1]), 1.0)

    hmask = (tags_i[:, :, None] == np.arange(T)[None, None, :]).astype(bf16)

    shared = dict(bd=bd, bdf=bdf, init=init, biasall=biasall, bias1=bias1,
                  trv=trv)
    in_maps = []
    for c in range(NC):
        sl = slice(c * BC, (c + 1) * BC)
        ftp = np.ascontiguousarray(ftp_full[:, :, :, sl]).reshape(128, Q * L * BC)
        cnt = np.ascontiguousarray(
            counts[sl].reshape(BC, NCK, 128).transpose(2, 1, 0)
        ).reshape(128, NCK * BC)
        im = {"ftp": ftp, "cnt": cnt,
              "hm": np.ascontiguousarray(hmask[sl]).reshape(BC, S * T),
              "ff": feats[sl].astype(bf16).reshape(BC, S * T)}
        im.update(shared)
        in_maps.append(im)
    return in_maps


_NC_CACHE = {}


def _get_program():
    if "nc" not in _NC_CACHE:
        _NC_CACHE["nc"] = _build_program()
    return _NC_CACHE["nc"]


def kernel(feats, transitions, start_tag, tags, mask_x, len_seq):
    feats = np.asarray(feats, dtype=np.float32)
    transitions = np.asarray(transitions, dtype=np.float32)
    start_tag = np.asarray(start_tag, dtype=np.float32)
    tags_np = np.asarray(tags)

    in_maps = _host_prep(feats, transitions, start_tag, tags_np)
    nc = _get_program()
    res = run_bass_kernel_spmd(nc, in_maps, list(range(NC)))
    out = np.concatenate([res.results[i]["out"][:, 0] for i in range(NC)])
    return out.astype(np.float32)


# revision 15
# speedup vs baseline: 6.9440x; 1.0066x over previous
"""CRF log-loss kernel for TRN2, data-parallel over batch on 8 NeuronCores.

Forward algorithm restructured for latency hiding:
  * The S=512-step sequence is split into NSEG=4 segments of 128 steps.
    Segment products are joined with rank-1 cross-approximation seams
    (error ~ (lambda2/lambda1)^128, far below tolerance):
        Z ~= (r3.c2)(g2.c1)(g1.c0) / (sum c2)(sum c1)
    where c_q are forward chains over segments 0..2 and g/r are backward
    chains over segments 1..3, all recursions of the same per-step cost and
    mutually independent -> 128 sequential rounds instead of 512.
  * Each round runs Q=3 paired chains: one 128x128 matmul against a
    resident block-diagonal stationary (fwd transition matrix in the top-left
    64x64, transposed one in the bottom-right) + one DVE multiply with the
    exp'd emissions.  Forward chain state lives in partitions 0-63, backward
    in 64-127.
  * exp(feats) runs on the scalar engine with host-calibrated per-chunk bias
    constants (no device-side renorm feedback), reading host-pre-transposed
    bf16 feats -> no DMA transposes, no serialization with the chain.
  * Gold score: transition+start terms as <per-example pair-count vector,
    [trans; start; start]> via 33 PSUM-accumulated matmuls on otherwise-idle
    PE cycles; emission term as one-hot x feats multiply-accumulate on the
    otherwise-idle gpsimd engine.
"""
import numpy as np
import ml_dtypes
from contextlib import ExitStack

import concourse.bass as bass
import concourse.bacc as bacc
import concourse.tile as tile
import concourse.mybir as mybir
from concourse.bass_utils import run_bass_kernel_spmd

bf16 = ml_dtypes.bfloat16
f32 = mybir.dt.float32
f16 = mybir.dt.float16
bf16d = mybir.dt.bfloat16
u16 = mybir.dt.uint16

B, S, T = 1024, 512, 64
NC = 8
BC = B // NC            # 128 examples per core
NSEG = 4
L = S // NSEG           # 128 rounds
Q = NSEG - 1            # 3 paired fwd/bwd chains
GRP = 8                 # rounds per exp group (= beta chunk size in steps)
NGRP = L // GRP         # 16 groups per chain
CHUNK = 8               # beta granularity in steps
NCH = S // CHUNK        # 64 chunks
NCV = T * T + 2 * T     # 4224: transition pairs + start-tag(first) + start-tag(last)
NCK = NCV // 128        # 33 contraction chunks for the gold count-matmul
NWIN = 16               # emission windows
WSZ = S * T // NWIN     # 2048 elements per window

AF = mybir.ActivationFunctionType
ALU = mybir.AluOpType


def _build_program():
    nc = bacc.Bacc("TRN2", target_bir_lowering=False, debug=False, num_devices=NC)

    ft_d = nc.dram_tensor("ftp", [128, Q * L * BC], bf16d, kind="ExternalInput")
    cnt_d = nc.dram_tensor("cnt", [128, NCK * BC], f16, kind="ExternalInput")
    trv_d = nc.dram_tensor("trv", [128, NCK], f16, kind="ExternalInput")
    hm_d = nc.dram_tensor("hm", [BC, S * T], bf16d, kind="ExternalInput")
    ff_d = nc.dram_tensor("ff", [BC, S * T], bf16d, kind="ExternalInput")
    bd_d = nc.dram_tensor("bd", [128, 128], bf16d, kind="ExternalInput")
    bdf_d = nc.dram_tensor("bdf", [128, T], bf16d, kind="ExternalInput")
    init_d = nc.dram_tensor("init", [128, Q * 128], bf16d, kind="ExternalInput")
    biasall_d = nc.dram_tensor("biasall", [128, Q * NGRP], f32, kind="ExternalInput")
    bias1_d = nc.dram_tensor("bias1", [BC, 1], f32, kind="ExternalInput")
    out_d = nc.dram_tensor("out", [BC, 1], f32, kind="ExternalOutput")

    with tile.TileContext(nc) as tc, ExitStack() as ctx:
        cpool = ctx.enter_context(tc.tile_pool(name="const", bufs=1))
        hmpool = ctx.enter_context(tc.tile_pool(name="hmw", bufs=2))
        ffpool = ctx.enter_context(tc.tile_pool(name="ffw", bufs=2))
        prpool = ctx.enter_context(tc.tile_pool(name="prw", bufs=2))
        scpool = ctx.enter_context(tc.tile_pool(name="scratch", bufs=2))
        ftpools = [ctx.enter_context(tc.tile_pool(name=f"ft{q}", bufs=2))
                   for q in range(Q)]
        etpools = [ctx.enter_context(tc.tile_pool(name=f"et{q}", bufs=2))
                   for q in range(Q)]
        stpools = [ctx.enter_context(tc.tile_pool(name=f"st{q}", bufs=2))
                   for q in range(Q)]
        pspools = [ctx.enter_context(tc.tile_pool(name=f"ps{q}", bufs=2, space="PSUM"))
                   for q in range(Q)]
        psfpool = ctx.enter_context(tc.tile_pool(name="psf", bufs=1, space="PSUM"))
        psdpool = ctx.enter_context(tc.tile_pool(name="psd", bufs=1, space="PSUM"))

        # ---- startup-critical DMAs first: first feats groups + chain consts;
        # gold-path consts ride later on the queue ----
        ft0 = [None] * Q
        for q in range(Q):
            ft = ftpools[q].tile([128, GRP * BC], bf16d, name=f"ftp{q}")
            nc.sync.dma_start(ft[:, :], ft_d[:, q * (L * BC):q * (L * BC) + GRP * BC])
            ft0[q] = ft
        biasall_s = cpool.tile([128, Q * NGRP], f32)
        nc.sync.dma_start(biasall_s[:, :], biasall_d[:, :])
        init_s = cpool.tile([128, Q * 128], bf16d)
        nc.sync.dma_start(init_s[:, :], init_d[:, :])
        bd_s = cpool.tile([128, 128], bf16d)
        nc.sync.dma_start(bd_s[:, :], bd_d[:, :])
        bdf_s = cpool.tile([128, T], bf16d)
        nc.sync.dma_start(bdf_s[:, :], bdf_d[:, :])
        bias1_s = cpool.tile([BC, 1], f32)
        nc.sync.dma_start(bias1_s[:, :], bias1_d[:, :])
        cnt_s = cpool.tile([128, NCK * BC], f16)
        trv_s = cpool.tile([128, NCK], f16)

        ones64 = cpool.tile([T, 1], bf16d)
        nc.vector.memset(ones64[:, :], 1.0)
        emis4 = cpool.tile([BC, NWIN], f32)
        ascr = cpool.tile([BC, WSZ], bf16d)
        psd = psdpool.tile([128, 512], f32)

        # ---- main rounds ----
        etts = [None] * Q
        stprev = [None] * Q
        win_done = 0

        def emit_window(w):
            hm = hmpool.tile([BC, WSZ], bf16d)
            nc.sync.dma_start(hm[:, :], hm_d[:, w * WSZ:(w + 1) * WSZ])
            ffw = ffpool.tile([BC, WSZ], bf16d)
            nc.sync.dma_start(ffw[:, :], ff_d[:, w * WSZ:(w + 1) * WSZ])
            prod = prpool.tile([BC, WSZ], bf16d)
            nc.gpsimd.tensor_tensor(prod[:, :], hm[:, :], ffw[:, :], ALU.mult)
            nc.scalar.activation(ascr[:, :], prod[:, :], AF.Copy,
                                 accum_out=emis4[:, w:w + 1])

        for r in range(L):
            if r % GRP == 0:
                g = r // GRP
                for q in range(Q):
                    if g == 0:
                        ft = ft0[q]
                    else:
                        ft = ftpools[q].tile([128, GRP * BC], bf16d, name=f"ftp{q}")
                        base = q * (L * BC) + g * (GRP * BC)
                        nc.sync.dma_start(ft[:, :], ft_d[:, base:base + GRP * BC])
                    ett = etpools[q].tile([128, GRP * BC], bf16d)
                    nc.scalar.activation(ett[:, :], ft[:, :], AF.Exp,
                                         bias=biasall_s[:, q * NGRP + g:q * NGRP + g + 1],
                                         scale=1.0)
                    etts[q] = ett
                if g == 1:
                    nc.sync.dma_start(cnt_s[:, :], cnt_d[:, :])
                    nc.sync.dma_start(trv_s[:, :], trv_d[:, :])
                # spread emission windows across the run (pool + scalar)
                if win_done < NWIN:
                    emit_window(win_done)
                    win_done += 1
            if 24 <= r < 24 + NCK:
                j = r - 24
                nc.tensor.matmul(psd[:, 8:9], cnt_s[:, j * BC:(j + 1) * BC],
                                 trv_s[:, j:j + 1], start=(j == 0),
                                 stop=(j == NCK - 1))
            sl = (r % GRP) * BC
            for q in range(Q):
                st = stpools[q].tile([128, BC], bf16d)
                if r == 0:
                    nc.vector.tensor_tensor(
                        st[:, :], init_s[:, q * 128:(q + 1) * 128],
                        etts[q][:, sl:sl + BC], ALU.mult)
                else:
                    ps = pspools[q].tile([128, 512], f32)
                    nc.tensor.matmul(ps[:, 0:BC], bd_s[:, :], stprev[q][:, :],
                                     start=True, stop=True)
                    nc.vector.tensor_tensor(
                        st[:, :], ps[:, 0:BC], etts[q][:, sl:sl + BC], ALU.mult)
                stprev[q] = st

        # ---- finals: g_q = T^T h_q placed in partitions 0-63, then the
        # per-example seam products z_q = g_q * c_q ----
        zs = []
        for q in range(Q):
            psf = psfpool.tile([128, 512], f32)
            nc.tensor.matmul(psf[0:T, 0:BC], bdf_s[:, :], stprev[q][:, :],
                             start=True, stop=True)
            z = scpool.tile([T, BC], bf16d, name=f"z{q}")
            nc.vector.tensor_tensor(z[:, :], psf[0:T, 0:BC],
                                    stprev[q][0:T, :], ALU.mult)
            zs.append(z)

        # ---- seam dots: column sums via matmul against ones ----
        dot_srcs = [zs[2][:, :], zs[1][:, :], zs[0][:, :],
                    stprev[2][0:T, :], stprev[1][0:T, :]]
        for j, src in enumerate(dot_srcs):
            nc.tensor.matmul(psd[:, j:j + 1], src, ones64[:, :],
                             start=True, stop=True)
        lns = scpool.tile([128, 5], f32)
        nc.scalar.activation(lns[:, :], psd[:, 0:5], AF.Ln)

        # ---- gold combine ----
        emisum = scpool.tile([BC, 1], f32)
        nc.vector.tensor_reduce(emisum[:, :], emis4[:, :],
                                axis=mybir.AxisListType.X, op=ALU.add)
        goldcol = scpool.tile([BC, 1], f32)
        nc.vector.tensor_add(goldcol[:, :], emisum[:, :], psd[:, 8:9])

        # ---- assemble: logZ = lnA+lnB+lnC-lnD-lnE + bias1 ; out = logZ-gold
        t1 = scpool.tile([BC, 1], f32)
        nc.vector.tensor_add(t1[:, :], lns[:, 0:1], lns[:, 1:2])
        t2 = scpool.tile([BC, 1], f32)
        nc.vector.tensor_add(t2[:, :], t1[:, :], lns[:, 2:3])
        t3 = scpool.tile([BC, 1], f32)
        nc.vector.tensor_sub(t3[:, :], t2[:, :], lns[:, 3:4])
        t4 = scpool.tile([BC, 1], f32)
        nc.vector.tensor_sub(t4[:, :], t3[:, :], lns[:, 4:5])
        t5 = scpool.tile([BC, 1], f32)
        nc.vector.tensor_add(t5[:, :], t4[:, :], bias1_s[:, :])
        lout = scpool.tile([BC, 1], f32)
        nc.vector.tensor_sub(lout[:, :], t5[:, :], goldcol[:, :])
        nc.sync.dma_start(out_d[:, :], lout[:, :])

    nc.compile()
    return nc


def _calibrate_beta(feats, transitions, start_tag, n_cal=8):
    """Per-chunk mean log-growth of the forward recursion, from a few
    examples, used as compile-free device bias constants."""
    Tm = np.exp(transitions.astype(np.float64))
    idx = np.linspace(0, B - 1, n_cal).astype(np.int64)
    u = np.tile(np.exp(start_tag.astype(np.float64))[None, :], (n_cal, 1))
    growth = np.zeros((n_cal, S))
    f = feats[idx].astype(np.float64)
    for s in range(S):
        u2 = np.exp(f[:, s, :]) * (u @ Tm.T)
        z = u2.sum(axis=1)
        growth[:, s] = np.log(z)
        u = u2 / z[:, None]
    g = growth.mean(axis=0)
    beta = -g.reshape(NCH, CHUNK).mean(axis=1)  # [NCH] per chunk
    return np.repeat(beta, CHUNK)               # [S] per step


def _host_prep(feats, transitions, start_tag, tags):
    """Shared (cross-core) constants + per-core tensors."""
    trans64 = transitions.astype(np.float64)
    Tm = np.exp(trans64)                       # T[j,k] = exp(trans[j,k])
    beta = _calibrate_beta(feats, transitions, start_tag)

    # block-diag stationary: BD[k,j]=T[j,k] (fwd), BD[64+k,64+j]=T[k,j] (bwd)
    bd = np.zeros((128, 128), dtype=np.float64)
    bd[:T, :T] = Tm.T
    bd[T:, T:] = Tm
    bd = bd.astype(bf16)
    # final bwd matmul: out[j] = sum_k T[k,j] h[k], j in partitions 0-63
    bdf = np.zeros((128, T), dtype=np.float64)
    bdf[T:, :] = Tm
    bdf = bdf.astype(bf16)

    # init tiles: top = (T @ u_start) replicated, bottom = p0 replicated
    u0 = np.exp(start_tag.astype(np.float64))
    pfin = Tm[T - 1, :]                        # exp(trans[63, :])
    init = np.zeros((128, Q * 128), dtype=np.float64)
    for q in range(Q):
        top = Tm @ (u0 if q == 0 else np.ones(T))
        bot = pfin if q == Q - 1 else np.ones(T)
        init[:T, q * 128:(q + 1) * 128] = top[:, None]
        init[T:, q * 128:(q + 1) * 128] = bot[:, None]
    init = init.astype(bf16)

    # per-(q, group) exp bias: top rows get the fwd chunk's beta, bottom rows
    # the bwd chunk's
    biasall = np.zeros((128, Q * NGRP), dtype=np.float32)
    for q in range(Q):
        for g in range(NGRP):
            cf = q * NGRP + g                  # fwd chunk index
            cb = (q + 2) * NGRP - 1 - g        # bwd chunk index
            biasall[:T, q * NGRP + g] = beta[cf * CHUNK]
            biasall[T:, q * NGRP + g] = beta[cb * CHUNK]
    bias1 = np.full((BC, 1), -beta.sum(), dtype=np.float32)


    # transposed emissions, round-major: ftp[p, q, r, b]
    #   p in 0..63  (tag): feats[b, q*128+r, tag]
    #   p in 64..127     : feats[b, (q+2)*128-1-r, tag]
    fs = np.ascontiguousarray(feats.transpose(1, 2, 0))  # [S, T, B]
    fwd = fs[:Q * L].reshape(Q, L, T, B).transpose(2, 0, 1, 3)
    bwd = fs[L:].reshape(Q, L, T, B)[:, ::-1].transpose(2, 0, 1, 3)
    ftp_full = np.concatenate([fwd, bwd], axis=0).astype(bf16)  # [128,Q,L,B]

    # gold-score inputs: per-example counts against [trans; start; start],
    # plus one-hot emission mask
    tags_i = tags.astype(np.int64)
    vec = np.concatenate([transitions.reshape(-1), start_tag, start_tag])
    trv = vec.astype(np.float16).reshape(NCK, 128).T.copy()     # [128, NCK]
    counts = np.zeros((B, NCV), dtype=np.float16)
    bidx = np.repeat(np.arange(B), S - 1)
    pairs = (tags_i[:, :S - 1] * T + tags_i[:, 1:]).reshape(-1)
    np.add.at(counts, (bidx, pairs), 1.0)
    np.add.at(counts, (np.arange(B), T * T + tags_i[:, 0]), 1.0)
    np.add.at(counts, (np.arange(B), T * T + T + tags_i[:, S - 1]), 1.0)

    hmask = (tags_i[:, :, None] == np.arange(T)[None, None, :]).astype(bf16)

    shared = dict(bd=bd, bdf=bdf, init=init, biasall=biasall, bias1=bias1,
                  trv=trv)
    in_maps = []
    for c in range(NC):
        sl = slice(c * BC, (c + 1) * BC)
        ftp = np.ascontiguousarray(ftp_full[:, :, :, sl]).reshape(128, Q * L * BC)
        cnt = np.ascontiguousarray(
            counts[sl].reshape(BC, NCK, 128).transpose(2, 1, 0)
        ).reshape(128, NCK * BC)
        im = {"ftp": ftp, "cnt": cnt,
              "hm": np.ascontiguousarray(hmask[sl]).reshape(BC, S * T),
              "ff": feats[sl].astype(bf16).reshape(BC, S * T)}
        im.update(shared)
        in_maps.append(im)
    return in_maps


_NC_CACHE = {}


def _get_program():
    if "nc" not in _NC_CACHE:
        _NC_CACHE["nc"] = _build_program()
    return _NC_CACHE["nc"]


def kernel(feats, transitions, start_tag, tags, mask_x, len_seq):
    feats = np.asarray(feats, dtype=np.float32)
    transitions = np.asarray(transitions, dtype=np.float32)
    start_tag = np.asarray(start_tag, dtype=np.float32)
    tags_np = np.asarray(tags)

    in_maps = _host_prep(feats, transitions, start_tag, tags_np)
    nc = _get_program()
    res = run_bass_kernel_spmd(nc, in_maps, list(range(NC)))
    out = np.concatenate([res.results[i]["out"][:, 0] for i in range(NC)])
    return out.astype(np.float32)


# revision 16
# speedup vs baseline: 7.2199x; 1.0397x over previous
"""CRF log-loss kernel for TRN2, data-parallel over batch on 8 NeuronCores.

Forward algorithm restructured for latency hiding:
  * The S=512-step sequence is split into NSEG=4 segments of 128 steps.
    Segment products are joined with rank-1 cross-approximation seams
    (error ~ (lambda2/lambda1)^128, far below tolerance):
        Z ~= (r3.c2)(g2.c1)(g1.c0) / (sum c2)(sum c1)
    where c_q are forward chains over segments 0..2 and g/r are backward
    chains over segments 1..3, all recursions of the same per-step cost and
    mutually independent -> 128 sequential rounds instead of 512.
  * Each round runs Q=3 paired chains: one 128x128 matmul against a
    resident block-diagonal stationary (fwd transition matrix in the top-left
    64x64, transposed one in the bottom-right) + one DVE multiply with the
    exp'd emissions.  Forward chain state lives in partitions 0-63, backward
    in 64-127.
  * exp(feats) runs on the scalar engine with host-calibrated per-chunk bias
    constants (no device-side renorm feedback), reading host-pre-transposed
    bf16 feats -> no DMA transposes, no serialization with the chain.
  * Gold score: transition+start terms as <per-example pair-count vector,
    [trans; start; start]> via 33 PSUM-accumulated matmuls on otherwise-idle
    PE cycles; emission term as one-hot x feats multiply-accumulate on the
    otherwise-idle gpsimd engine.
"""
import numpy as np
import ml_dtypes
from contextlib import ExitStack

import concourse.bass as bass
import concourse.bacc as bacc
import concourse.tile as tile
import concourse.mybir as mybir
from concourse.bass_utils import run_bass_kernel_spmd

bf16 = ml_dtypes.bfloat16
fp8 = ml_dtypes.float8_e4m3
f32 = mybir.dt.float32
f16 = mybir.dt.float16
bf16d = mybir.dt.bfloat16
u16 = mybir.dt.uint16
f8 = mybir.dt.float8e4

B, S, T = 1024, 512, 64
NC = 8
BC = B // NC            # 128 examples per core
NSEG = 4
L = S // NSEG           # 128 rounds
Q = NSEG - 1            # 3 paired fwd/bwd chains
GRP = 16                # rounds per exp group (= beta chunk size in steps)
NGRP = L // GRP         # 8 groups per chain
CHUNK = 16              # beta granularity in steps
NCH = S // CHUNK        # 64 chunks
NCV = T * T + 2 * T     # 4224: transition pairs + start-tag(first) + start-tag(last)
NCK = NCV // 128        # 33 contraction chunks for the gold count-matmul
NWIN = 16               # emission windows
WSZ = S * T // NWIN     # 2048 elements per window

AF = mybir.ActivationFunctionType
ALU = mybir.AluOpType


def _build_program():
    nc = bacc.Bacc("TRN2", target_bir_lowering=False, debug=False, num_devices=NC)

    ft_d = nc.dram_tensor("ftp", [128, Q * L * BC], f8, kind="ExternalInput")
    cnt_d = nc.dram_tensor("cnt", [128, NCK * BC], f16, kind="ExternalInput")
    trv_d = nc.dram_tensor("trv", [128, NCK], f16, kind="ExternalInput")
    hm_d = nc.dram_tensor("hm", [BC, S * T], f8, kind="ExternalInput")
    ff_d = nc.dram_tensor("ff", [BC, S * T], f8, kind="ExternalInput")
    bd_d = nc.dram_tensor("bd", [128, 128], bf16d, kind="ExternalInput")
    bdf_d = nc.dram_tensor("bdf", [128, T], bf16d, kind="ExternalInput")
    init_d = nc.dram_tensor("init", [128, Q * 128], bf16d, kind="ExternalInput")
    biasall_d = nc.dram_tensor("biasall", [128, Q * NGRP], f32, kind="ExternalInput")
    bias1_d = nc.dram_tensor("bias1", [BC, 1], f32, kind="ExternalInput")
    out_d = nc.dram_tensor("out", [BC, 1], f32, kind="ExternalOutput")

    with tile.TileContext(nc) as tc, ExitStack() as ctx:
        cpool = ctx.enter_context(tc.tile_pool(name="const", bufs=1))
        hmpool = ctx.enter_context(tc.tile_pool(name="hmw", bufs=2))
        ffpool = ctx.enter_context(tc.tile_pool(name="ffw", bufs=2))
        prpool = ctx.enter_context(tc.tile_pool(name="prw", bufs=2))
        scpool = ctx.enter_context(tc.tile_pool(name="scratch", bufs=2))
        ftpools = [ctx.enter_context(tc.tile_pool(name=f"ft{q}", bufs=2))
                   for q in range(Q)]
        etpools = [ctx.enter_context(tc.tile_pool(name=f"et{q}", bufs=2))
                   for q in range(Q)]
        stpools = [ctx.enter_context(tc.tile_pool(name=f"st{q}", bufs=2))
                   for q in range(Q)]
        pspools = [ctx.enter_context(tc.tile_pool(name=f"ps{q}", bufs=2, space="PSUM"))
                   for q in range(Q)]
        psfpool = ctx.enter_context(tc.tile_pool(name="psf", bufs=1, space="PSUM"))
        psdpool = ctx.enter_context(tc.tile_pool(name="psd", bufs=1, space="PSUM"))

        # ---- startup-critical DMAs first: first feats groups + chain consts;
        # gold-path consts ride later on the queue ----
        ft0 = [None] * Q
        for q in range(Q):
            ft = ftpools[q].tile([128, GRP * BC], f8, name=f"ftp{q}")
            nc.sync.dma_start(ft[:, :], ft_d[:, q * (L * BC):q * (L * BC) + GRP * BC])
            ft0[q] = ft
        biasall_s = cpool.tile([128, Q * NGRP], f32)
        nc.sync.dma_start(biasall_s[:, :], biasall_d[:, :])
        init_s = cpool.tile([128, Q * 128], bf16d)
        nc.sync.dma_start(init_s[:, :], init_d[:, :])
        bd_s = cpool.tile([128, 128], bf16d)
        nc.sync.dma_start(bd_s[:, :], bd_d[:, :])
        bdf_s = cpool.tile([128, T], bf16d)
        nc.sync.dma_start(bdf_s[:, :], bdf_d[:, :])
        bias1_s = cpool.tile([BC, 1], f32)
        nc.sync.dma_start(bias1_s[:, :], bias1_d[:, :])
        cnt_s = cpool.tile([128, NCK * BC], f16)
        trv_s = cpool.tile([128, NCK], f16)

        ones64 = cpool.tile([T, 1], bf16d)
        nc.vector.memset(ones64[:, :], 1.0)
        emis4 = cpool.tile([BC, NWIN], f32)
        ascr = cpool.tile([BC, WSZ], bf16d)
        psd = psdpool.tile([128, 512], f32)

        # ---- main rounds ----
        etts = [None] * Q
        stprev = [None] * Q
        win_done = 0

        def emit_window(w):
            hm = hmpool.tile([BC, WSZ], f8)
            nc.sync.dma_start(hm[:, :], hm_d[:, w * WSZ:(w + 1) * WSZ])
            ffw = ffpool.tile([BC, WSZ], f8)
            nc.sync.dma_start(ffw[:, :], ff_d[:, w * WSZ:(w + 1) * WSZ])
            prod = prpool.tile([BC, WSZ], bf16d)
            nc.gpsimd.tensor_tensor(prod[:, :], hm[:, :], ffw[:, :], ALU.mult)
            nc.scalar.activation(ascr[:, :], prod[:, :], AF.Copy,
                                 accum_out=emis4[:, w:w + 1])

        for r in range(L):
            if r % GRP == 0:
                g = r // GRP
                for q in range(Q):
                    if g == 0:
                        ft = ft0[q]
                    else:
                        ft = ftpools[q].tile([128, GRP * BC], f8, name=f"ftp{q}")
                        base = q * (L * BC) + g * (GRP * BC)
                        nc.sync.dma_start(ft[:, :], ft_d[:, base:base + GRP * BC])
                    ett = etpools[q].tile([128, GRP * BC], bf16d)
                    nc.scalar.activation(ett[:, :], ft[:, :], AF.Exp,
                                         bias=biasall_s[:, q * NGRP + g:q * NGRP + g + 1],
                                         scale=1.0)
                    etts[q] = ett
                if g == 1:
                    nc.sync.dma_start(cnt_s[:, :], cnt_d[:, :])
                    nc.sync.dma_start(trv_s[:, :], trv_d[:, :])
                # spread emission windows across the run (pool + scalar)
                for _ in range(2):
                    if win_done < NWIN:
                        emit_window(win_done)
                        win_done += 1
            if 24 <= r < 24 + NCK:
                j = r - 24
                nc.tensor.matmul(psd[:, 8:9], cnt_s[:, j * BC:(j + 1) * BC],
                                 trv_s[:, j:j + 1], start=(j == 0),
                                 stop=(j == NCK - 1))
            sl = (r % GRP) * BC
            for q in range(Q):
                st = stpools[q].tile([128, BC], bf16d)
                if r == 0:
                    nc.vector.tensor_tensor(
                        st[:, :], init_s[:, q * 128:(q + 1) * 128],
                        etts[q][:, sl:sl + BC], ALU.mult)
                else:
                    ps = pspools[q].tile([128, 512], f32)
                    nc.tensor.matmul(ps[:, 0:BC], bd_s[:, :], stprev[q][:, :],
                                     start=True, stop=True)
                    nc.vector.tensor_tensor(
                        st[:, :], ps[:, 0:BC], etts[q][:, sl:sl + BC], ALU.mult)
                stprev[q] = st

        # ---- finals: g_q = T^T h_q placed in partitions 0-63, then the
        # per-example seam products z_q = g_q * c_q ----
        zs = []
        for q in range(Q):
            psf = psfpool.tile([128, 512], f32)
            nc.tensor.matmul(psf[0:T, 0:BC], bdf_s[:, :], stprev[q][:, :],
                             start=True, stop=True)
            z = scpool.tile([T, BC], bf16d, name=f"z{q}")
            nc.vector.tensor_tensor(z[:, :], psf[0:T, 0:BC],
                                    stprev[q][0:T, :], ALU.mult)
            zs.append(z)

        # ---- seam dots: column sums via matmul against ones ----
        dot_srcs = [zs[2][:, :], zs[1][:, :], zs[0][:, :],
                    stprev[2][0:T, :], stprev[1][0:T, :]]
        for j, src in enumerate(dot_srcs):
            nc.tensor.matmul(psd[:, j:j + 1], src, ones64[:, :],
                             start=True, stop=True)
        lns = scpool.tile([128, 5], f32)
        nc.scalar.activation(lns[:, :], psd[:, 0:5], AF.Ln)

        # ---- gold combine ----
        emisum = scpool.tile([BC, 1], f32)
        nc.vector.tensor_reduce(emisum[:, :], emis4[:, :],
                                axis=mybir.AxisListType.X, op=ALU.add)
        goldcol = scpool.tile([BC, 1], f32)
        nc.vector.tensor_add(goldcol[:, :], emisum[:, :], psd[:, 8:9])

        # ---- assemble: logZ = lnA+lnB+lnC-lnD-lnE + bias1 ; out = logZ-gold
        t1 = scpool.tile([BC, 1], f32)
        nc.vector.tensor_add(t1[:, :], lns[:, 0:1], lns[:, 1:2])
        t2 = scpool.tile([BC, 1], f32)
        nc.vector.tensor_add(t2[:, :], t1[:, :], lns[:, 2:3])
        t3 = scpool.tile([BC, 1], f32)
        nc.vector.tensor_sub(t3[:, :], t2[:, :], lns[:, 3:4])
        t4 = scpool.tile([BC, 1], f32)
        nc.vector.tensor_sub(t4[:, :], t3[:, :], lns[:, 4:5])
        t5 = scpool.tile([BC, 1], f32)
        nc.vector.tensor_add(t5[:, :], t4[:, :], bias1_s[:, :])
        lout = scpool.tile([BC, 1], f32)
        nc.vector.tensor_sub(lout[:, :], t5[:, :], goldcol[:, :])
        nc.sync.dma_start(out_d[:, :], lout[:, :])

    nc.compile()
    return nc


def _calibrate_beta(feats, transitions, start_tag, n_cal=8):
    """Per-chunk mean log-growth of the forward recursion, from a few
    examples, used as compile-free device bias constants."""
    Tm = np.exp(transitions.astype(np.float64))
    idx = np.linspace(0, B - 1, n_cal).astype(np.int64)
    u = np.tile(np.exp(start_tag.astype(np.float64))[None, :], (n_cal, 1))
    growth = np.zeros((n_cal, S))
    f = feats[idx].astype(np.float64)
    for s in range(S):
        u2 = np.exp(f[:, s, :]) * (u @ Tm.T)
        z = u2.sum(axis=1)
        growth[:, s] = np.log(z)
        u = u2 / z[:, None]
    g = growth.mean(axis=0)
    beta = -g.reshape(NCH, CHUNK).mean(axis=1)  # [NCH] per chunk
    return np.repeat(beta, CHUNK)               # [S] per step


def _host_prep(feats, transitions, start_tag, tags):
    """Shared (cross-core) constants + per-core tensors."""
    trans64 = transitions.astype(np.float64)
    Tm = np.exp(trans64)                       # T[j,k] = exp(trans[j,k])
    beta = _calibrate_beta(feats, transitions, start_tag)

    # block-diag stationary: BD[k,j]=T[j,k] (fwd), BD[64+k,64+j]=T[k,j] (bwd)
    bd = np.zeros((128, 128), dtype=np.float64)
    bd[:T, :T] = Tm.T
    bd[T:, T:] = Tm
    bd = bd.astype(bf16)
    # final bwd matmul: out[j] = sum_k T[k,j] h[k], j in partitions 0-63
    bdf = np.zeros((128, T), dtype=np.float64)
    bdf[T:, :] = Tm
    bdf = bdf.astype(bf16)

    # init tiles: top = (T @ u_start) replicated, bottom = p0 replicated
    u0 = np.exp(start_tag.astype(np.float64))
    pfin = Tm[T - 1, :]                        # exp(trans[63, :])
    init = np.zeros((128, Q * 128), dtype=np.float64)
    for q in range(Q):
        top = Tm @ (u0 if q == 0 else np.ones(T))
        bot = pfin if q == Q - 1 else np.ones(T)
        init[:T, q * 128:(q + 1) * 128] = top[:, None]
        init[T:, q * 128:(q + 1) * 128] = bot[:, None]
    init = init.astype(bf16)

    # per-(q, group) exp bias: top rows get the fwd chunk's beta, bottom rows
    # the bwd chunk's
    biasall = np.zeros((128, Q * NGRP), dtype=np.float32)
    for q in range(Q):
        for g in range(NGRP):
            cf = q * NGRP + g                  # fwd chunk index
            cb = (q + 2) * NGRP - 1 - g        # bwd chunk index
            biasall[:T, q * NGRP + g] = beta[cf * CHUNK]
            biasall[T:, q * NGRP + g] = beta[cb * CHUNK]
    bias1 = np.full((BC, 1), -beta.sum(), dtype=np.float32)


    # transposed emissions, round-major: ftp[p, q, r, b]
    #   p in 0..63  (tag): feats[b, q*128+r, tag]
    #   p in 64..127     : feats[b, (q+2)*128-1-r, tag]
    fs = np.ascontiguousarray(feats.transpose(1, 2, 0))  # [S, T, B]
    fwd = fs[:Q * L].reshape(Q, L, T, B).transpose(2, 0, 1, 3)
    bwd = fs[L:].reshape(Q, L, T, B)[:, ::-1].transpose(2, 0, 1, 3)
    ftp_full = np.concatenate([fwd, bwd], axis=0).astype(fp8)  # [128,Q,L,B]

    # gold-score inputs: per-example counts against [trans; start; start],
    # plus one-hot emission mask
    tags_i = tags.astype(np.int64)
    vec = np.concatenate([transitions.reshape(-1), start_tag, start_tag])
    trv = vec.astype(np.float16).reshape(NCK, 128).T.copy()     # [128, NCK]
    counts = np.zeros((B, NCV), dtype=np.float16)
    bidx = np.repeat(np.arange(B), S - 1)
    pairs = (tags_i[:, :S - 1] * T + tags_i[:, 1:]).reshape(-1)
    np.add.at(counts, (bidx, pairs), 1.0)
    np.add.at(counts, (np.arange(B), T * T + tags_i[:, 0]), 1.0)
    np.add.at(counts, (np.arange(B), T * T + T + tags_i[:, S - 1]), 1.0)

    hmask = (tags_i[:, :, None] == np.arange(T)[None, None, :]).astype(fp8)

    shared = dict(bd=bd, bdf=bdf, init=init, biasall=biasall, bias1=bias1,
                  trv=trv)
    in_maps = []
    for c in range(NC):
        sl = slice(c * BC, (c + 1) * BC)
        ftp = np.ascontiguousarray(ftp_full[:, :, :, sl]).reshape(128, Q * L * BC)
        cnt = np.ascontiguousarray(
            counts[sl].reshape(BC, NCK, 128).transpose(2, 1, 0)
        ).reshape(128, NCK * BC)
        im = {"ftp": ftp, "cnt": cnt,
              "hm": np.ascontiguousarray(hmask[sl]).reshape(BC, S * T),
              "ff": feats[sl].astype(fp8).reshape(BC, S * T)}
        im.update(shared)
        in_maps.append(im)
    return in_maps


_NC_CACHE = {}


def _get_program():
    if "nc" not in _NC_CACHE:
        _NC_CACHE["nc"] = _build_program()
    return _NC_CACHE["nc"]


def kernel(feats, transitions, start_tag, tags, mask_x, len_seq):
    feats = np.asarray(feats, dtype=np.float32)
    transitions = np.asarray(transitions, dtype=np.float32)
    start_tag = np.asarray(start_tag, dtype=np.float32)
    tags_np = np.asarray(tags)

    in_maps = _host_prep(feats, transitions, start_tag, tags_np)
    nc = _get_program()
    res = run_bass_kernel_spmd(nc, in_maps, list(range(NC)))
    out = np.concatenate([res.results[i]["out"][:, 0] for i in range(NC)])
    return out.astype(np.float32)
